# revision 19
# baseline (speedup 1.0000x reference)
"""Distributed Trainium2 kernel for nn_CompareLoss (8 NeuronCores), v4.

Math (validated against the reference):
  z = [strong; weak] (2B x D), s = z/||z||, logits(i,j) = (s_i.s_j)/tau.
  The whole loss reduces to exps of the [2P x 2N] matrix L with rows
  [spos; wpos] and cols [sneg; wneg]:
    loss1 row sums   = row sums of exp(L)               (all 2P rows)
    loss2 "col" sums = col sums of exp(L[:P, :])        (spos rows only)
  plus pair logits p_i = s_spos_i.s_wpos_i, q_j = s_sneg_j.s_wneg_j.
  Host does the final ln(S+e^p)-p reduction in float64 (tiny).  This
  exploits sim symmetry: the baseline recomputed the neg-row x spos-col
  blocks (25.2M exps); here 16.8M only.

Sharding: 2D grid, 4 row-groups x 2 col-groups.  Core (r,g) owns 512
pos-pair rows (spos/wpos slab r) and 1024 neg cols (sneg/wneg group g,
rolled by -256r so each core owns a disjoint 256-slice of neg pairs).
Columns per core, in six 512-col chunks:
  [spos | sn0 | sn1 | wn0 | wn1 | wpos]  (3072 total)

Device pipeline:
  - 6 chunk DMAs, one descriptor per partition (host packs each chunk
    [h0|h1]-contiguous), same queue, NO dep chains: packets stream
    back-to-back at full BW and chunk completions stagger for the
    square chase.  iv/obb ride the gpsimd queue (Pool triggers ~25ns).
  - Column sumsq: DVE squares + h-presum -> ONE ones-window matmul per
    chunk (OZ1, value 1.0 - tau is NOT in the lhsT so nothing waits the
    iv DMA).  rn = Exp(-0.5*Ln(ssq) - 0.5*ln(tau)): Ln and Exp need one
    table load each way, the exp-set load sits between them
    (structural); dummy Ln at t=0 preloads the ln set during DMA.  The
    tau bias is a [6,1] AP built from a K=1 matmul + ACT Ln + mul.
  - rn broadcast WITHOUT DMA: one-hot [6x128] matmuls (OBB) replicate
    rn_t row c across partitions into [128,512] psum pieces; DVE
    normalizes zt against psum directly, writing fp8e4 ztn (the PSUM
    operand forces DVE 1x anyway, so fp8 output is free).
  - 16 main tiles [128,1024]: 2 fp8 DoubleRow matmuls each (K=256 in
    one pass via the [128,2,512] h-layout) + ACT Exp with fused row-sum
    (accum_out -> ACC[128,16]).  spos exps write real fp8 values; ONE
    DoubleRow window matmul per spos tile (OZD: kt0 hot at row 2u, kt1
    at 2u+1) col-sums both 512-chunks into cs rows 0..3.  Colsums are
    deferred into the wpos phase so the PE is never the per-tile
    bottleneck while its clock ramps.  Pair logits land in cs rows 4/5.
  - Outputs: ACC [128,16] and cs[0:6] f32; host finishes in f64.
  - PSUM: sps 1 bank (warm/ssq/cs) + rnb 2x1 + mains 2x2 = 7 of 8.
"""

import numpy as np

B = 4096
D = 256
P = 2048
NCORES = 8
RG = 4                    # row groups (pos-pair slabs of 512)
CG = 2                    # col groups (neg slabs of 1024)
SLAB = P // RG            # 512 pos pairs per row-group
CGN = P // CG             # 1024 negs per col-group
NCH = 6                   # 512-col chunks: [spos|sn0|sn1|wn0|wn1|wpos]
NCOL = 512 * NCH

_CACHE: dict = {}


def _build_nc():
    import concourse.bacc as bacc
    import concourse.tile as tile
    from concourse import mybir

    f32 = mybir.dt.float32
    f16 = mybir.dt.float16
    f8 = mybir.dt.float8e4
    DR = mybir.MatmulPerfMode.DoubleRow
    EXP = mybir.ActivationFunctionType.Exp
    LN = mybir.ActivationFunctionType.Ln

    nc = bacc.Bacc("TRN2", target_bir_lowering=False, debug=False,
                   num_devices=NCORES)
    zt_d = nc.dram_tensor("zt", [128, 2 * NCOL], f16, kind="ExternalInput")
    iv_d = nc.dram_tensor("tauv", [1, 1], f32, kind="ExternalInput")
    obb_d = nc.dram_tensor("obb", [6, 6 * 128], f16, kind="ExternalInput")
    acc_d = nc.dram_tensor("acc", [128, 16], f32, kind="ExternalOutput")
    cs_d = nc.dram_tensor("cs", [6, 512], f32, kind="ExternalOutput")

    # chunk-major host layout: row p = [c0h0|c0h1|c1h0|c1h1|...]
    zt6 = zt_d[:, :].rearrange("p (c h n) -> p c h n", c=NCH, h=2)

    with tile.TileContext(nc) as tc:
        with (
            tc.tile_pool(name="const", bufs=1) as constp,
            tc.tile_pool(name="big", bufs=1) as bigp,
            tc.tile_pool(name="work", bufs=3) as workp,
            tc.tile_pool(name="esc", bufs=8) as escp,
            tc.tile_pool(name="sps", bufs=1, space="PSUM") as sps,
            tc.tile_pool(name="rnbp", bufs=2, space="PSUM") as rnbp,
            tc.tile_pool(name="mps", bufs=2, space="PSUM") as mps,
        ):
            # ---------------- gpsimd: dummy + aux DMAs + consts --------
            dum1 = constp.tile([1, 1], f16)
            nc.gpsimd.memset(dum1[:], 1.0)
            ivt = constp.tile([1, 1], f32)
            nc.gpsimd.dma_start(ivt[:], iv_d[:])
            OBB = constp.tile([6, 6 * 128], f16)
            nc.gpsimd.dma_start(OBB[:], obb_d[:])

            # dummy Ln: preloads the natural_log table set during DMA
            dumo = constp.tile([1, 1], f32)
            nc.scalar.activation(dumo[:], dum1[:], LN)

            ones16_1 = constp.tile([1, 128], f16)
            nc.gpsimd.memset(ones16_1[:], 1.0)
            # OZ1: ones window (ssq reductions + pair sums), hot col 6
            OZ1 = constp.tile([128, 12], f16)
            nc.gpsimd.memset(OZ1[:], 0.0)
            nc.gpsimd.memset(OZ1[:, 6:7], 1.0)
            # OZD: DoubleRow colsum windows, M=32 pad (DR rejects tiny
            # M): kt=0 hot at window pos m0, kt=1 at m0+1
            OZD = constp.tile([128, 128], f8)
            nc.gpsimd.memset(OZD[:], 0.0)
            nc.gpsimd.memset(OZD[:, 32:33], 1.0)
            nc.gpsimd.memset(OZD[:, 97:98], 1.0)
            OZD3 = OZD[:, :].rearrange("p (k c) -> p k c", k=2)

            # ---------------- input DMAs (sync queue, unchained) -------
            zt4 = bigp.tile([128, NCH, 2, 512], f16)
            for c in range(NCH):
                nc.sync.dma_start(zt4[:, c], zt6[:, c])

            # ---------------- tau bias: -0.5*ln(tau) as [6,1] ----------
            iv16 = constp.tile([1, 1], f16)
            nc.vector.tensor_copy(iv16[:], ivt[:])
            tau_bc = rnbp.tile([128, 512], f32, tag="rnb", name="tau_bc")
            nc.tensor.matmul(tau_bc[0:6, 0:1], ones16_1[0:1, 0:6],
                             iv16[0:1, 0:1], start=True, stop=True)
            lntau = constp.tile([6, 1], f32)
            nc.scalar.activation(lntau[:], tau_bc[0:6, 0:1], LN)
            bias_t = constp.tile([6, 1], f32)
            nc.scalar.mul(bias_t[:], lntau[:], -0.5)

            # ---------------- column sumsq -> rn ----------------
            sq4 = bigp.tile([128, NCH, 2, 512], f16)
            hsq = bigp.tile([128, NCH, 512], f16)
            ssq = sps.tile([6, 512], f32, tag="sps")
            for c in range(NCH):
                nc.vector.tensor_mul(sq4[:, c], zt4[:, c], zt4[:, c])
                nc.vector.tensor_add(hsq[:, c], sq4[:, c, 0], sq4[:, c, 1])
                nc.tensor.matmul(ssq[0:6, 0:512], OZ1[:, 6 - c:12 - c],
                                 hsq[:, c], start=(c == 0),
                                 stop=(c == NCH - 1))

            lnt = constp.tile([6, 512], f32)
            nc.scalar.activation(lnt[:], ssq[0:6, :], LN)
            rn_t = constp.tile([6, 512], f16)
            nc.scalar.activation(rn_t[:], lnt[:], EXP, scale=-0.5,
                                 bias=bias_t[:])

            # ---------------- rn broadcast + normalize (fp8) ----------
            ztn = bigp.tile([128, NCH, 2, 512], f8)
            for c in range(NCH):
                rp = rnbp.tile([128, 512], f32, tag="rnb", name=f"rnb{c}")
                nc.tensor.matmul(rp[:], OBB[:, 128 * c:128 * c + 128],
                                 rn_t[0:6, :], start=True, stop=True)
                for h in range(2):
                    nc.vector.tensor_mul(ztn[:, c, h], zt4[:, c, h], rp[:])

            # ---------------- main tiles ----------------
            ACC = constp.tile([128, 16], f32)
            escJ = constp.tile([128, 1024], f8)    # wpos exp sink
            cs = sps.tile([32, 512], f32, tag="sps")
            cs_started = [False]
            escs = []

            def cs_mm(row, rhs_ap, stop=False):
                nc.tensor.matmul(cs[0:6, 0:rhs_ap.shape[-1]],
                                 OZ1[:, 6 - row:12 - row], rhs_ap,
                                 start=not cs_started[0], stop=stop,
                                 skip_group_check=True)
                cs_started[0] = True

            def cs_mm_dr(u, esc, stop=False):
                # one DoubleRow matmul col-sums BOTH 512-chunks of an
                # esc tile into cs rows 2u (kt=0) and 2u+1 (kt=1)
                e3 = esc[:, :].rearrange("p (k n) -> p k n", k=2)
                m0 = 2 * u
                nc.tensor.matmul(cs[0:32, 0:512],
                                 OZD3[:, :, 32 - m0:64 - m0], e3[:, :, :],
                                 start=not cs_started[0], stop=stop,
                                 perf_mode=DR, skip_group_check=True)
                cs_started[0] = True

            def main_tile(T, lhs_ch, lhs_off, u, is_spos):
                ps = mps.tile([128, 1024], f32, tag="mps", name=f"mm{T}")
                for c2 in range(2):
                    nc.tensor.matmul(
                        ps[:, 512 * c2:512 * c2 + 512],
                        ztn[:, lhs_ch, :, lhs_off:lhs_off + 128],
                        ztn[:, 1 + 2 * u + c2], start=True, stop=True,
                        perf_mode=DR)
                if is_spos:
                    esc = escp.tile([128, 1024], f8, tag="esc",
                                    name=f"esc{T}")
                    nc.scalar.activation(esc[:], ps[:], EXP,
                                         accum_out=ACC[:, T:T + 1])
                    escs.append(esc)
                else:
                    nc.scalar.activation(escJ[:], ps[:], EXP,
                                         accum_out=ACC[:, T:T + 1])

            for t in range(4):
                for u in range(2):
                    main_tile(2 * t + u, 0, 128 * t, u, True)

            # pair logits (DVE products + window matmuls into cs 4/5)
            pr_pos = workp.tile([128, 2, 512], f16, tag="pr")
            nc.vector.tensor_mul(pr_pos[:], ztn[:, 0], ztn[:, 5])
            pr_neg = workp.tile([128, 2, 256], f16, tag="pr")
            nc.vector.tensor_mul(pr_neg[:], ztn[:, 1, :, 0:256],
                                 ztn[:, 3, :, 0:256])

            for t in range(4):
                for u in range(2):
                    T = 8 + 2 * t + u
                    main_tile(T, 5, 128 * t, u, False)
                    k = T - 8
                    cs_mm_dr(k % 2, escs[k])
            cs_mm(4, pr_pos[:, 0, :])
            cs_mm(4, pr_pos[:, 1, :])
            cs_mm(5, pr_neg[:, 0, :])
            cs_mm(5, pr_neg[:, 1, :], stop=True)

            # ---------------- outputs ----------------
            csb = constp.tile([6, 512], f32)
            nc.vector.tensor_copy(csb[:], cs[0:6, :])
            nc.sync.dma_start(acc_d[:], ACC[:])
            nc.sync.dma_start(cs_d[:], csb[:])

    nc.compile()
    return nc


def get_nc():
    if "nc" not in _CACHE:
        _CACHE["nc"] = _build_nc()
    return _CACHE["nc"]


def make_in_maps(strong: np.ndarray, weak: np.ndarray, temp: np.ndarray):
    """Host-side sharding: slice + roll + transpose (pure data movement)."""
    tauv = np.asarray(temp, np.float32).reshape(1, 1)
    obb = np.zeros((6, 6 * 128), np.float16)
    for j in range(NCH):
        obb[j, 128 * j:128 * j + 128] = 1.0
    in_maps = []
    for r in range(RG):
        for g in range(CG):
            spos = strong[SLAB * r:SLAB * r + SLAB]
            wpos = weak[SLAB * r:SLAB * r + SLAB]
            sneg = np.roll(strong[P + CGN * g:P + CGN * g + CGN],
                           -256 * r, axis=0)
            wneg = np.roll(weak[P + CGN * g:P + CGN * g + CGN],
                           -256 * r, axis=0)
            cols = np.concatenate([spos, sneg, wneg, wpos], axis=0)
            zt16 = cols.T.astype(np.float16)              # [256, 3072]
            # [h,p,c,n] -> [p, c, h, n] chunk-major contiguous rows
            ztd = np.ascontiguousarray(
                zt16.reshape(2, 128, NCH, 512).transpose(1, 2, 0, 3)
                .reshape(128, 2 * NCOL))
            in_maps.append({"zt": ztd, "tauv": tauv, "obb": obb})
    return in_maps


def kernel(inputs, strong_inputs, targets, num_pos, temperature):
    assert int(num_pos) == P
    strong = np.ascontiguousarray(np.asarray(strong_inputs, dtype=np.float32))
    weak = np.ascontiguousarray(np.asarray(inputs, dtype=np.float32))
    temp = np.asarray(temperature, dtype=np.float32).reshape(1, 1)

    from concourse.bass_utils import run_bass_kernel_spmd

    nc = get_nc()
    in_maps = make_in_maps(strong, weak, temp)
    res = run_bass_kernel_spmd(nc, in_maps, core_ids=list(range(NCORES)))
    return finish_host(res.results)


def finish_host(results):
    """Final ln(S + e^p) - p reduction in float64 on the host."""
    S1s = np.zeros((RG, SLAB))
    S1w = np.zeros((RG, SLAB))
    CA = np.zeros((CG, CGN))
    CB = np.zeros((CG, CGN))
    pos_l = np.zeros((RG, SLAB))
    neg_l = np.zeros((CG, CGN))
    for r in range(RG):
        for g in range(CG):
            res = results[CG * r + g]
            acc = np.asarray(res["acc"], np.float64)     # [128, 16]
            cs = np.asarray(res["cs"], np.float64)       # [6, 512]
            for t in range(4):
                sl = slice(128 * t, 128 * t + 128)
                S1s[r, sl] += acc[:, 2 * t] + acc[:, 2 * t + 1]
                S1w[r, sl] += acc[:, 8 + 2 * t] + acc[:, 8 + 2 * t + 1]
            CA[g] += np.roll(cs[0:2].reshape(CGN), 256 * r)
            CB[g] += np.roll(cs[2:4].reshape(CGN), 256 * r)
            if g == 0:
                pos_l[r] = cs[4]
            neg_l[g, 256 * r:256 * r + 256] = cs[5, 0:256]
    p = pos_l.reshape(-1)
    q = neg_l.reshape(-1)
    ep, eq = np.exp(p), np.exp(q)
    total = (np.sum(np.log(S1s.reshape(-1) + ep) - p)
             + np.sum(np.log(S1w.reshape(-1) + ep) - p)
             + np.sum(np.log(CA.reshape(-1) + eq) - q)
             + np.sum(np.log(CB.reshape(-1) + eq) - q))
    return np.float32(total / (2 * B))


# revision 27
# speedup vs baseline: 1.0661x; 1.0661x over previous
"""Distributed Trainium2 kernel for nn_CompareLoss (8 NeuronCores), v4.

Math (validated against the reference):
  z = [strong; weak] (2B x D), s = z/||z||, logits(i,j) = (s_i.s_j)/tau.
  The whole loss reduces to exps of the [2P x 2N] matrix L with rows
  [spos; wpos] and cols [sneg; wneg]:
    loss1 row sums   = row sums of exp(L)               (all 2P rows)
    loss2 "col" sums = col sums of exp(L[:P, :])        (spos rows only)
  plus pair logits p_i = s_spos_i.s_wpos_i, q_j = s_sneg_j.s_wneg_j.
  Host does the final ln(S+e^p)-p reduction in float64 (tiny).  This
  exploits sim symmetry: the baseline recomputed the neg-row x spos-col
  blocks (25.2M exps); here 16.8M only.

Sharding: 2D grid, 4 row-groups x 2 col-groups.  Core (r,g) owns 512
pos-pair rows (spos/wpos slab r) and 1024 neg cols (sneg/wneg group g,
rolled by -256r so each core owns a disjoint 256-slice of neg pairs).
Columns per core, in six 512-col chunks:
  [spos | sn0 | sn1 | wn0 | wn1 | wpos]  (3072 total)

Device pipeline:
  - 6 chunk DMAs, one descriptor per partition (host packs each chunk
    [h0|h1]-contiguous), same queue, NO dep chains: packets stream
    back-to-back at full BW and chunk completions stagger for the
    square chase.  iv/obb ride the gpsimd queue (Pool triggers ~25ns).
  - Column sumsq: DVE squares + h-presum -> ONE ones-window matmul per
    chunk (OZ1, value 1.0 - tau is NOT in the lhsT so nothing waits the
    iv DMA).  rn = Exp(-0.5*Ln(ssq) - 0.5*ln(tau)): Ln and Exp need one
    table load each way, the exp-set load sits between them
    (structural); dummy Ln at t=0 preloads the ln set during DMA.  The
    tau bias is a [6,1] AP built from a K=1 matmul + ACT Ln + mul.
  - rn broadcast WITHOUT DMA: one-hot [6x128] matmuls (OBB) replicate
    rn_t row c across partitions into [128,512] psum pieces; DVE
    normalizes zt against psum directly, writing fp8e4 ztn (the PSUM
    operand forces DVE 1x anyway, so fp8 output is free).
  - 16 main tiles [128,1024]: 2 fp8 DoubleRow matmuls each (K=256 in
    one pass via the [128,2,512] h-layout) + ACT Exp with fused row-sum
    (accum_out -> ACC[128,16]).  spos exps write real fp8 values; ONE
    DoubleRow window matmul per spos tile (OZD: kt0 hot at row 2u, kt1
    at 2u+1) col-sums both 512-chunks into cs rows 0..3.  Colsums are
    deferred into the wpos phase so the PE is never the per-tile
    bottleneck while its clock ramps.  Pair logits land in cs rows 4/5.
  - Outputs: ACC [128,16] and cs[0:6] f32; host finishes in f64.
  - PSUM: sps 1 bank (warm/ssq/cs) + rnb 2x1 + mains 2x2 = 7 of 8.
"""

import numpy as np

B = 4096
D = 256
P = 2048
NCORES = 8
RG = 4                    # row groups (pos-pair slabs of 512)
CG = 2                    # col groups (neg slabs of 1024)
SLAB = P // RG            # 512 pos pairs per row-group
CGN = P // CG             # 1024 negs per col-group
NCH = 6                   # 512-col chunks: [spos|sn0|sn1|wn0|wn1|wpos]
NCOL = 512 * NCH

_CACHE: dict = {}


def _build_nc():
    import concourse.bacc as bacc
    import concourse.tile as tile
    from concourse import mybir

    f32 = mybir.dt.float32
    f16 = mybir.dt.float16
    f8 = mybir.dt.float8e4
    DR = mybir.MatmulPerfMode.DoubleRow
    EXP = mybir.ActivationFunctionType.Exp
    LN = mybir.ActivationFunctionType.Ln

    SQUARE = mybir.ActivationFunctionType.Square

    nc = bacc.Bacc("TRN2", target_bir_lowering=False, debug=False,
                   num_devices=NCORES)
    zt_d = nc.dram_tensor("zt", [128, 2 * NCOL], f8, kind="ExternalInput")
    iv_d = nc.dram_tensor("tauv", [1, 1], f32, kind="ExternalInput")
    obb_d = nc.dram_tensor("obb", [6, 6 * 128], f16, kind="ExternalInput")
    acc_d = nc.dram_tensor("acc", [128, 16], f32, kind="ExternalOutput")
    cs_d = nc.dram_tensor("cs", [6, 512], f32, kind="ExternalOutput")

    # chunk-major host layout: row p = [c0h0|c0h1|c1h0|c1h1|...]
    zt6 = zt_d[:, :].rearrange("p (c h n) -> p c h n", c=NCH, h=2)

    with tile.TileContext(nc) as tc:
        with (
            tc.tile_pool(name="const", bufs=1) as constp,
            tc.tile_pool(name="big", bufs=1) as bigp,
            tc.tile_pool(name="work", bufs=3) as workp,
            tc.tile_pool(name="esc", bufs=8) as escp,
            tc.tile_pool(name="sps", bufs=1, space="PSUM") as sps,
            tc.tile_pool(name="rnbp", bufs=2, space="PSUM") as rnbp,
            tc.tile_pool(name="mps", bufs=2, space="PSUM") as mps,
        ):
            # ---------------- input DMAs (two queues, unchained) -------
            # fp8 input halves the stream time; even chunks trigger on
            # sync, odd on the scalar queue (ahead of the dummy Ln and
            # its table load) so all six triggers land early.
            zt4 = bigp.tile([128, NCH, 2, 512], f8)
            for c in range(1, NCH, 2):
                nc.scalar.dma_start(zt4[:, c], zt6[:, c])
            for c in range(0, NCH, 2):
                nc.sync.dma_start(zt4[:, c], zt6[:, c])

            # ---------------- gpsimd: dummy + aux DMAs + consts --------
            dum1 = constp.tile([1, 1], f16)
            nc.gpsimd.memset(dum1[:], 1.0)
            ivt = constp.tile([1, 1], f32)
            nc.gpsimd.dma_start(ivt[:], iv_d[:])
            OBB = constp.tile([6, 6 * 128], f16)
            nc.gpsimd.dma_start(OBB[:], obb_d[:])

            # dummy Ln: preloads the natural_log table set during DMA
            dumo = constp.tile([1, 1], f32)
            nc.scalar.activation(dumo[:], dum1[:], LN)

            ones16_1 = constp.tile([1, 128], f16)
            nc.gpsimd.memset(ones16_1[:], 1.0)
            # OZ1: ones window (ssq reductions + pair sums), hot col 6
            OZ1 = constp.tile([128, 12], f16)
            nc.gpsimd.memset(OZ1[:], 0.0)
            nc.gpsimd.memset(OZ1[:, 6:7], 1.0)
            # OZD: DoubleRow colsum windows, M=32 pad (DR rejects tiny
            # M): kt=0 hot at window pos m0, kt=1 at m0+1
            OZD = constp.tile([128, 128], f8)
            nc.gpsimd.memset(OZD[:], 0.0)
            nc.gpsimd.memset(OZD[:, 32:33], 1.0)
            nc.gpsimd.memset(OZD[:, 97:98], 1.0)
            OZD3 = OZD[:, :].rearrange("p (k c) -> p k c", k=2)

            # ---------------- tau bias: -0.5*ln(tau) as [6,1] ----------
            iv16 = constp.tile([1, 1], f16)
            nc.vector.tensor_copy(iv16[:], ivt[:])
            tau_bc = rnbp.tile([128, 512], f32, tag="rnb", name="tau_bc")
            nc.tensor.matmul(tau_bc[0:6, 0:1], ones16_1[0:1, 0:6],
                             iv16[0:1, 0:1], start=True, stop=True)
            lntau = constp.tile([6, 1], f32)
            nc.scalar.activation(lntau[:], tau_bc[0:6, 0:1], LN)
            bias_t = constp.tile([6, 1], f32)
            nc.scalar.mul(bias_t[:], lntau[:], -0.5)

            # ---------------- column sumsq -> rn ----------------
            # squares split across idle ACT (even chunks; Square is in
            # the natural_log set, no extra table load) and DVE (odd),
            # h-presum on DVE, one ones-window matmul per chunk.
            sq4 = bigp.tile([128, NCH, 2, 512], f16)
            hsq = bigp.tile([128, NCH, 512], f16)
            ssq = sps.tile([6, 512], f32, tag="sps")
            for c in range(NCH):
                if c % 2 == 0:
                    nc.scalar.activation(sq4[:, c], zt4[:, c], SQUARE)
                else:
                    nc.vector.tensor_mul(sq4[:, c], zt4[:, c], zt4[:, c])
                nc.vector.tensor_add(hsq[:, c], sq4[:, c, 0], sq4[:, c, 1])
                nc.tensor.matmul(ssq[0:6, 0:512], OZ1[:, 6 - c:12 - c],
                                 hsq[:, c], start=(c == 0),
                                 stop=(c == NCH - 1))

            lnt = constp.tile([6, 512], f32)
            nc.scalar.activation(lnt[:], ssq[0:6, :], LN)
            rn_t = constp.tile([6, 512], f16)
            nc.scalar.activation(rn_t[:], lnt[:], EXP, scale=-0.5,
                                 bias=bias_t[:])

            # ---------------- rn broadcast + normalize (fp8) ----------
            ztn = bigp.tile([128, NCH, 2, 512], f8)
            for c in range(NCH):
                rp = rnbp.tile([128, 512], f32, tag="rnb", name=f"rnb{c}")
                nc.tensor.matmul(rp[:], OBB[:, 128 * c:128 * c + 128],
                                 rn_t[0:6, :], start=True, stop=True)
                for h in range(2):
                    nc.vector.tensor_mul(ztn[:, c, h], zt4[:, c, h], rp[:])

            # ---------------- main tiles ----------------
            ACC = constp.tile([128, 16], f32)
            escJ = constp.tile([128, 1024], f8)    # wpos exp sink
            cs = sps.tile([32, 512], f32, tag="sps")
            cs_started = [False]
            escs = []

            def cs_mm(row, rhs_ap, stop=False):
                nc.tensor.matmul(cs[0:6, 0:rhs_ap.shape[-1]],
                                 OZ1[:, 6 - row:12 - row], rhs_ap,
                                 start=not cs_started[0], stop=stop,
                                 skip_group_check=True)
                cs_started[0] = True

            def cs_mm_dr(u, esc, stop=False):
                # one DoubleRow matmul col-sums BOTH 512-chunks of an
                # esc tile into cs rows 2u (kt=0) and 2u+1 (kt=1)
                e3 = esc[:, :].rearrange("p (k n) -> p k n", k=2)
                m0 = 2 * u
                nc.tensor.matmul(cs[0:32, 0:512],
                                 OZD3[:, :, 32 - m0:64 - m0], e3[:, :, :],
                                 start=not cs_started[0], stop=stop,
                                 perf_mode=DR, skip_group_check=True)
                cs_started[0] = True

            def main_tile(T, lhs_ch, lhs_off, u, is_spos):
                ps = mps.tile([128, 1024], f32, tag="mps", name=f"mm{T}")
                for c2 in range(2):
                    nc.tensor.matmul(
                        ps[:, 512 * c2:512 * c2 + 512],
                        ztn[:, lhs_ch, :, lhs_off:lhs_off + 128],
                        ztn[:, 1 + 2 * u + c2], start=True, stop=True,
                        perf_mode=DR)
                if is_spos:
                    esc = escp.tile([128, 1024], f8, tag="esc",
                                    name=f"esc{T}")
                    nc.scalar.activation(esc[:], ps[:], EXP,
                                         accum_out=ACC[:, T:T + 1])
                    escs.append((esc, u))
                else:
                    nc.scalar.activation(escJ[:], ps[:], EXP,
                                         accum_out=ACC[:, T:T + 1])

            # all u=0 tiles first: they only need chunks 0..2 normalized
            for u in range(2):
                for t in range(4):
                    main_tile(2 * t + u, 0, 128 * t, u, True)

            # pair logits (DVE products + window matmuls into cs 4/5)
            pr_pos = workp.tile([128, 2, 512], f16, tag="pr")
            nc.vector.tensor_mul(pr_pos[:], ztn[:, 0], ztn[:, 5])
            pr_neg = workp.tile([128, 2, 256], f16, tag="pr")
            nc.vector.tensor_mul(pr_neg[:], ztn[:, 1, :, 0:256],
                                 ztn[:, 3, :, 0:256])

            k = [0]
            for u in range(2):
                for t in range(4):
                    main_tile(8 + 2 * t + u, 5, 128 * t, u, False)
                    e, eu = escs[k[0]]
                    cs_mm_dr(eu, e)
                    k[0] += 1
            cs_mm(4, pr_pos[:, 0, :])
            cs_mm(4, pr_pos[:, 1, :])
            cs_mm(5, pr_neg[:, 0, :])
            cs_mm(5, pr_neg[:, 1, :], stop=True)

            # ---------------- outputs ----------------
            csb = constp.tile([6, 512], f32)
            nc.vector.tensor_copy(csb[:], cs[0:6, :])
            nc.sync.dma_start(acc_d[:], ACC[:])
            nc.sync.dma_start(cs_d[:], csb[:])

    nc.compile()
    return nc


def get_nc():
    if "nc" not in _CACHE:
        _CACHE["nc"] = _build_nc()
    return _CACHE["nc"]


def make_in_maps(strong: np.ndarray, weak: np.ndarray, temp: np.ndarray):
    """Host-side sharding: slice + roll + transpose (pure data movement)."""
    tauv = np.asarray(temp, np.float32).reshape(1, 1)
    obb = np.zeros((6, 6 * 128), np.float16)
    for j in range(NCH):
        obb[j, 128 * j:128 * j + 128] = 1.0
    in_maps = []
    for r in range(RG):
        for g in range(CG):
            spos = strong[SLAB * r:SLAB * r + SLAB]
            wpos = weak[SLAB * r:SLAB * r + SLAB]
            sneg = np.roll(strong[P + CGN * g:P + CGN * g + CGN],
                           -256 * r, axis=0)
            wneg = np.roll(weak[P + CGN * g:P + CGN * g + CGN],
                           -256 * r, axis=0)
            import ml_dtypes
            cols = np.concatenate([spos, sneg, wneg, wpos], axis=0)
            zt8 = cols.T.astype(ml_dtypes.float8_e4m3fn)  # [256, 3072]
            # [h,p,c,n] -> [p, c, h, n] chunk-major contiguous rows
            ztd = np.ascontiguousarray(
                zt8.reshape(2, 128, NCH, 512).transpose(1, 2, 0, 3)
                .reshape(128, 2 * NCOL))
            in_maps.append({"zt": ztd, "tauv": tauv, "obb": obb})
    return in_maps


def kernel(inputs, strong_inputs, targets, num_pos, temperature):
    assert int(num_pos) == P
    strong = np.ascontiguousarray(np.asarray(strong_inputs, dtype=np.float32))
    weak = np.ascontiguousarray(np.asarray(inputs, dtype=np.float32))
    temp = np.asarray(temperature, dtype=np.float32).reshape(1, 1)

    from concourse.bass_utils import run_bass_kernel_spmd

    nc = get_nc()
    in_maps = make_in_maps(strong, weak, temp)
    res = run_bass_kernel_spmd(nc, in_maps, core_ids=list(range(NCORES)))
    return finish_host(res.results)


def finish_host(results):
    """Final ln(S + e^p) - p reduction in float64 on the host."""
    S1s = np.zeros((RG, SLAB))
    S1w = np.zeros((RG, SLAB))
    CA = np.zeros((CG, CGN))
    CB = np.zeros((CG, CGN))
    pos_l = np.zeros((RG, SLAB))
    neg_l = np.zeros((CG, CGN))
    for r in range(RG):
        for g in range(CG):
            res = results[CG * r + g]
            acc = np.asarray(res["acc"], np.float64)     # [128, 16]
            cs = np.asarray(res["cs"], np.float64)       # [6, 512]
            for t in range(4):
                sl = slice(128 * t, 128 * t + 128)
                S1s[r, sl] += acc[:, 2 * t] + acc[:, 2 * t + 1]
                S1w[r, sl] += acc[:, 8 + 2 * t] + acc[:, 8 + 2 * t + 1]
            CA[g] += np.roll(cs[0:2].reshape(CGN), 256 * r)
            CB[g] += np.roll(cs[2:4].reshape(CGN), 256 * r)
            if g == 0:
                pos_l[r] = cs[4]
            neg_l[g, 256 * r:256 * r + 256] = cs[5, 0:256]
    p = pos_l.reshape(-1)
    q = neg_l.reshape(-1)
    ep, eq = np.exp(p), np.exp(q)
    total = (np.sum(np.log(S1s.reshape(-1) + ep) - p)
             + np.sum(np.log(S1w.reshape(-1) + ep) - p)
             + np.sum(np.log(CA.reshape(-1) + eq) - q)
             + np.sum(np.log(CB.reshape(-1) + eq) - q))
    return np.float32(total / (2 * B))


# revision 30
# speedup vs baseline: 1.0692x; 1.0028x over previous
"""Distributed Trainium2 kernel for nn_CompareLoss (8 NeuronCores), v4.

Math (validated against the reference):
  z = [strong; weak] (2B x D), s = z/||z||, logits(i,j) = (s_i.s_j)/tau.
  The whole loss reduces to exps of the [2P x 2N] matrix L with rows
  [spos; wpos] and cols [sneg; wneg]:
    loss1 row sums   = row sums of exp(L)               (all 2P rows)
    loss2 "col" sums = col sums of exp(L[:P, :])        (spos rows only)
  plus pair logits p_i = s_spos_i.s_wpos_i, q_j = s_sneg_j.s_wneg_j.
  Host does the final ln(S+e^p)-p reduction in float64 (tiny).  This
  exploits sim symmetry: the baseline recomputed the neg-row x spos-col
  blocks (25.2M exps); here 16.8M only.

Sharding: 2D grid, 4 row-groups x 2 col-groups.  Core (r,g) owns 512
pos-pair rows (spos/wpos slab r) and 1024 neg cols (sneg/wneg group g,
rolled by -256r so each core owns a disjoint 256-slice of neg pairs).
Columns per core, in six 512-col chunks:
  [spos | sn0 | sn1 | wn0 | wn1 | wpos]  (3072 total)

Device pipeline:
  - 6 chunk DMAs, one descriptor per partition (host packs each chunk
    [h0|h1]-contiguous), same queue, NO dep chains: packets stream
    back-to-back at full BW and chunk completions stagger for the
    square chase.  iv/obb ride the gpsimd queue (Pool triggers ~25ns).
  - Column sumsq: DVE squares + h-presum -> ONE ones-window matmul per
    chunk (OZ1, value 1.0 - tau is NOT in the lhsT so nothing waits the
    iv DMA).  rn = Exp(-0.5*Ln(ssq) - 0.5*ln(tau)): Ln and Exp need one
    table load each way, the exp-set load sits between them
    (structural); dummy Ln at t=0 preloads the ln set during DMA.  The
    tau bias is a [6,1] AP built from a K=1 matmul + ACT Ln + mul.
  - rn broadcast WITHOUT DMA: one-hot [6x128] matmuls (OBB) replicate
    rn_t row c across partitions into [128,512] psum pieces; DVE
    normalizes zt against psum directly, writing fp8e4 ztn (the PSUM
    operand forces DVE 1x anyway, so fp8 output is free).
  - 16 main tiles [128,1024]: 2 fp8 DoubleRow matmuls each (K=256 in
    one pass via the [128,2,512] h-layout) + ACT Exp with fused row-sum
    (accum_out -> ACC[128,16]).  spos exps write real fp8 values; ONE
    DoubleRow window matmul per spos tile (OZD: kt0 hot at row 2u, kt1
    at 2u+1) col-sums both 512-chunks into cs rows 0..3.  Colsums are
    deferred into the wpos phase so the PE is never the per-tile
    bottleneck while its clock ramps.  Pair logits land in cs rows 4/5.
  - Outputs: ACC [128,16] and cs[0:6] f32; host finishes in f64.
  - PSUM: sps 1 bank (warm/ssq/cs) + rnb 2x1 + mains 2x2 = 7 of 8.
"""

import numpy as np

B = 4096
D = 256
P = 2048
NCORES = 8
RG = 4                    # row groups (pos-pair slabs of 512)
CG = 2                    # col groups (neg slabs of 1024)
SLAB = P // RG            # 512 pos pairs per row-group
CGN = P // CG             # 1024 negs per col-group
NCH = 6                   # 512-col chunks: [spos|sn0|sn1|wn0|wn1|wpos]
NCOL = 512 * NCH

_CACHE: dict = {}


def _build_nc():
    import concourse.bacc as bacc
    import concourse.tile as tile
    from concourse import mybir

    f32 = mybir.dt.float32
    f16 = mybir.dt.float16
    f8 = mybir.dt.float8e4
    DR = mybir.MatmulPerfMode.DoubleRow
    EXP = mybir.ActivationFunctionType.Exp
    LN = mybir.ActivationFunctionType.Ln

    SQUARE = mybir.ActivationFunctionType.Square

    nc = bacc.Bacc("TRN2", target_bir_lowering=False, debug=False,
                   num_devices=NCORES)
    zt_d = nc.dram_tensor("zt", [128, 2 * NCOL], f8, kind="ExternalInput")
    iv_d = nc.dram_tensor("tauv", [1, 1], f32, kind="ExternalInput")
    obb_d = nc.dram_tensor("obb", [6, 6 * 128], f16, kind="ExternalInput")
    acc_d = nc.dram_tensor("acc", [128, 16], f32, kind="ExternalOutput")
    cs_d = nc.dram_tensor("cs", [6, 512], f32, kind="ExternalOutput")

    # chunk-major host layout: row p = [c0h0|c0h1|c1h0|c1h1|...]
    zt6 = zt_d[:, :].rearrange("p (c h n) -> p c h n", c=NCH, h=2)

    with tile.TileContext(nc) as tc:
        with (
            tc.tile_pool(name="const", bufs=1) as constp,
            tc.tile_pool(name="big", bufs=1) as bigp,
            tc.tile_pool(name="work", bufs=3) as workp,
            tc.tile_pool(name="esc", bufs=8) as escp,
            tc.tile_pool(name="sps", bufs=1, space="PSUM") as sps,
            tc.tile_pool(name="rnbp", bufs=2, space="PSUM") as rnbp,
            tc.tile_pool(name="mps", bufs=2, space="PSUM") as mps,
        ):
            # ---------------- input DMAs (two queues, unchained) -------
            # fp8 input halves the stream time; even chunks trigger on
            # sync, odd on the scalar queue (ahead of the dummy Ln and
            # its table load) so all six triggers land early.
            # reversed order on the scalar queue so both queue tails land
            # mid-sequence: arrivals pair up as (c0,c5), (c2,c3), (c4,c1)
            zt4 = bigp.tile([128, NCH, 2, 512], f8)
            for c in (5, 3, 1):
                nc.scalar.dma_start(zt4[:, c], zt6[:, c])
            for c in (0, 2, 4):
                nc.sync.dma_start(zt4[:, c], zt6[:, c])

            # ---------------- gpsimd: dummy + aux DMAs + consts --------
            dum1 = constp.tile([1, 1], f16)
            nc.gpsimd.memset(dum1[:], 1.0)
            ivt = constp.tile([1, 1], f32)
            nc.gpsimd.dma_start(ivt[:], iv_d[:])
            OBB = constp.tile([6, 6 * 128], f16)
            nc.gpsimd.dma_start(OBB[:], obb_d[:])

            # dummy Ln: preloads the natural_log table set during DMA
            dumo = constp.tile([1, 1], f32)
            nc.scalar.activation(dumo[:], dum1[:], LN)

            ones16_1 = constp.tile([1, 128], f16)
            nc.gpsimd.memset(ones16_1[:], 1.0)
            # OZ1: ones window (ssq reductions + pair sums), hot col 6
            OZ1 = constp.tile([128, 12], f16)
            nc.gpsimd.memset(OZ1[:], 0.0)
            nc.gpsimd.memset(OZ1[:, 6:7], 1.0)
            # OZD: DoubleRow colsum windows, M=32 pad (DR rejects tiny
            # M): kt=0 hot at window pos m0, kt=1 at m0+1
            OZD = constp.tile([128, 128], f8)
            nc.gpsimd.memset(OZD[:], 0.0)
            nc.gpsimd.memset(OZD[:, 32:33], 1.0)
            nc.gpsimd.memset(OZD[:, 97:98], 1.0)
            OZD3 = OZD[:, :].rearrange("p (k c) -> p k c", k=2)

            # ---------------- tau bias: -0.5*ln(tau) as [6,1] ----------
            iv16 = constp.tile([1, 1], f16)
            nc.vector.tensor_copy(iv16[:], ivt[:])
            tau_bc = rnbp.tile([128, 512], f32, tag="rnb", name="tau_bc")
            nc.tensor.matmul(tau_bc[0:6, 0:1], ones16_1[0:1, 0:6],
                             iv16[0:1, 0:1], start=True, stop=True)
            lntau = constp.tile([6, 1], f32)
            nc.scalar.activation(lntau[:], tau_bc[0:6, 0:1], LN)
            bias_t = constp.tile([6, 1], f32)
            nc.scalar.mul(bias_t[:], lntau[:], -0.5)

            # ---------------- column sumsq -> rn ----------------
            # squares split: ACT takes c0,c2,c4,c1 (Square is in the
            # natural_log set, no extra table load; ACT is idle here),
            # DVE takes c5,c3 (emitted first so they aren't queued
            # behind hadds).  h-presum on DVE, one ones-window matmul
            # per chunk, all in expected-arrival order.
            ARR = (0, 5, 2, 3, 4, 1)
            sq4 = bigp.tile([128, NCH, 2, 512], f16)
            hsq = bigp.tile([128, NCH, 512], f16)
            ssq = sps.tile([6, 512], f32, tag="sps")
            for c in (5, 3):
                nc.vector.tensor_mul(sq4[:, c], zt4[:, c], zt4[:, c])
            for c in (0, 2, 4, 1):
                nc.scalar.activation(sq4[:, c], zt4[:, c], SQUARE)
            for i, c in enumerate(ARR):
                nc.vector.tensor_add(hsq[:, c], sq4[:, c, 0], sq4[:, c, 1])
                nc.tensor.matmul(ssq[0:6, 0:512], OZ1[:, 6 - c:12 - c],
                                 hsq[:, c], start=(i == 0),
                                 stop=(i == NCH - 1))

            lnt = constp.tile([6, 512], f32)
            nc.scalar.activation(lnt[:], ssq[0:6, :], LN)
            rn_t = constp.tile([6, 512], f16)
            nc.scalar.activation(rn_t[:], lnt[:], EXP, scale=-0.5,
                                 bias=bias_t[:])

            # ---------------- rn broadcast + normalize (fp8) ----------
            ztn = bigp.tile([128, NCH, 2, 512], f8)
            for c in range(NCH):
                rp = rnbp.tile([128, 512], f32, tag="rnb", name=f"rnb{c}")
                nc.tensor.matmul(rp[:], OBB[:, 128 * c:128 * c + 128],
                                 rn_t[0:6, :], start=True, stop=True)
                for h in range(2):
                    nc.vector.tensor_mul(ztn[:, c, h], zt4[:, c, h], rp[:])

            # ---------------- main tiles ----------------
            ACC = constp.tile([128, 16], f32)
            escJ = constp.tile([128, 1024], f8)    # wpos exp sink
            cs = sps.tile([32, 512], f32, tag="sps")
            cs_started = [False]
            escs = []

            def cs_mm(row, rhs_ap, stop=False):
                nc.tensor.matmul(cs[0:6, 0:rhs_ap.shape[-1]],
                                 OZ1[:, 6 - row:12 - row], rhs_ap,
                                 start=not cs_started[0], stop=stop,
                                 skip_group_check=True)
                cs_started[0] = True

            def cs_mm_dr(u, esc, stop=False):
                # one DoubleRow matmul col-sums BOTH 512-chunks of an
                # esc tile into cs rows 2u (kt=0) and 2u+1 (kt=1)
                e3 = esc[:, :].rearrange("p (k n) -> p k n", k=2)
                m0 = 2 * u
                nc.tensor.matmul(cs[0:32, 0:512],
                                 OZD3[:, :, 32 - m0:64 - m0], e3[:, :, :],
                                 start=not cs_started[0], stop=stop,
                                 perf_mode=DR, skip_group_check=True)
                cs_started[0] = True

            def main_tile(T, lhs_ch, lhs_off, u, is_spos):
                ps = mps.tile([128, 1024], f32, tag="mps", name=f"mm{T}")
                for c2 in range(2):
                    nc.tensor.matmul(
                        ps[:, 512 * c2:512 * c2 + 512],
                        ztn[:, lhs_ch, :, lhs_off:lhs_off + 128],
                        ztn[:, 1 + 2 * u + c2], start=True, stop=True,
                        perf_mode=DR)
                if is_spos:
                    esc = escp.tile([128, 1024], f8, tag="esc",
                                    name=f"esc{T}")
                    nc.scalar.activation(esc[:], ps[:], EXP,
                                         accum_out=ACC[:, T:T + 1])
                    escs.append((esc, u))
                else:
                    nc.scalar.activation(escJ[:], ps[:], EXP,
                                         accum_out=ACC[:, T:T + 1])

            # all u=0 tiles first: they only need chunks 0..2 normalized
            for u in range(2):
                for t in range(4):
                    main_tile(2 * t + u, 0, 128 * t, u, True)

            # pair logits (DVE products + window matmuls into cs 4/5)
            pr_pos = workp.tile([128, 2, 512], f16, tag="pr")
            nc.vector.tensor_mul(pr_pos[:], ztn[:, 0], ztn[:, 5])
            pr_neg = workp.tile([128, 2, 256], f16, tag="pr")
            nc.vector.tensor_mul(pr_neg[:], ztn[:, 1, :, 0:256],
                                 ztn[:, 3, :, 0:256])

            k = [0]
            for u in range(2):
                for t in range(4):
                    main_tile(8 + 2 * t + u, 5, 128 * t, u, False)
                    e, eu = escs[k[0]]
                    cs_mm_dr(eu, e, stop=(k[0] == 7))
                    k[0] += 1
                    if k[0] == 4:
                        # pair-sum matmuls mid-phase so the cs copy+DMA
                        # stay off the measured tail
                        cs_mm(4, pr_pos[:, 0, :])
                        cs_mm(4, pr_pos[:, 1, :])
                        cs_mm(5, pr_neg[:, 0, :])
                        cs_mm(5, pr_neg[:, 1, :])

            # ---------------- outputs ----------------
            csb = constp.tile([6, 512], f32)
            nc.vector.tensor_copy(csb[:], cs[0:6, :])
            nc.sync.dma_start(acc_d[:], ACC[:])
            nc.sync.dma_start(cs_d[:], csb[:])

    nc.compile()
    return nc


def get_nc():
    if "nc" not in _CACHE:
        _CACHE["nc"] = _build_nc()
    return _CACHE["nc"]


def make_in_maps(strong: np.ndarray, weak: np.ndarray, temp: np.ndarray):
    """Host-side sharding: slice + roll + transpose (pure data movement)."""
    tauv = np.asarray(temp, np.float32).reshape(1, 1)
    obb = np.zeros((6, 6 * 128), np.float16)
    for j in range(NCH):
        obb[j, 128 * j:128 * j + 128] = 1.0
    in_maps = []
    for r in range(RG):
        for g in range(CG):
            spos = strong[SLAB * r:SLAB * r + SLAB]
            wpos = weak[SLAB * r:SLAB * r + SLAB]
            sneg = np.roll(strong[P + CGN * g:P + CGN * g + CGN],
                           -256 * r, axis=0)
            wneg = np.roll(weak[P + CGN * g:P + CGN * g + CGN],
                           -256 * r, axis=0)
            import ml_dtypes
            cols = np.concatenate([spos, sneg, wneg, wpos], axis=0)
            zt8 = cols.T.astype(ml_dtypes.float8_e4m3fn)  # [256, 3072]
            # [h,p,c,n] -> [p, c, h, n] chunk-major contiguous rows
            ztd = np.ascontiguousarray(
                zt8.reshape(2, 128, NCH, 512).transpose(1, 2, 0, 3)
                .reshape(128, 2 * NCOL))
            in_maps.append({"zt": ztd, "tauv": tauv, "obb": obb})
    return in_maps


def kernel(inputs, strong_inputs, targets, num_pos, temperature):
    assert int(num_pos) == P
    strong = np.ascontiguousarray(np.asarray(strong_inputs, dtype=np.float32))
    weak = np.ascontiguousarray(np.asarray(inputs, dtype=np.float32))
    temp = np.asarray(temperature, dtype=np.float32).reshape(1, 1)

    from concourse.bass_utils import run_bass_kernel_spmd

    nc = get_nc()
    in_maps = make_in_maps(strong, weak, temp)
    res = run_bass_kernel_spmd(nc, in_maps, core_ids=list(range(NCORES)))
    return finish_host(res.results)


def finish_host(results):
    """Final ln(S + e^p) - p reduction in float64 on the host."""
    S1s = np.zeros((RG, SLAB))
    S1w = np.zeros((RG, SLAB))
    CA = np.zeros((CG, CGN))
    CB = np.zeros((CG, CGN))
    pos_l = np.zeros((RG, SLAB))
    neg_l = np.zeros((CG, CGN))
    for r in range(RG):
        for g in range(CG):
            res = results[CG * r + g]
            acc = np.asarray(res["acc"], np.float64)     # [128, 16]
            cs = np.asarray(res["cs"], np.float64)       # [6, 512]
            for t in range(4):
                sl = slice(128 * t, 128 * t + 128)
                S1s[r, sl] += acc[:, 2 * t] + acc[:, 2 * t + 1]
                S1w[r, sl] += acc[:, 8 + 2 * t] + acc[:, 8 + 2 * t + 1]
            CA[g] += np.roll(cs[0:2].reshape(CGN), 256 * r)
            CB[g] += np.roll(cs[2:4].reshape(CGN), 256 * r)
            if g == 0:
                pos_l[r] = cs[4]
            neg_l[g, 256 * r:256 * r + 256] = cs[5, 0:256]
    p = pos_l.reshape(-1)
    q = neg_l.reshape(-1)
    ep, eq = np.exp(p), np.exp(q)
    total = (np.sum(np.log(S1s.reshape(-1) + ep) - p)
             + np.sum(np.log(S1w.reshape(-1) + ep) - p)
             + np.sum(np.log(CA.reshape(-1) + eq) - q)
             + np.sum(np.log(CB.reshape(-1) + eq) - q))
    return np.float32(total / (2 * B))


# revision 34
# speedup vs baseline: 1.1219x; 1.0494x over previous
"""Distributed Trainium2 kernel for nn_CompareLoss (8 NeuronCores), v4.

Math (validated against the reference):
  z = [strong; weak] (2B x D), s = z/||z||, logits(i,j) = (s_i.s_j)/tau.
  The whole loss reduces to exps of the [2P x 2N] matrix L with rows
  [spos; wpos] and cols [sneg; wneg]:
    loss1 row sums   = row sums of exp(L)               (all 2P rows)
    loss2 "col" sums = col sums of exp(L[:P, :])        (spos rows only)
  plus pair logits p_i = s_spos_i.s_wpos_i, q_j = s_sneg_j.s_wneg_j.
  Host does the final ln(S+e^p)-p reduction in float64 (tiny).  This
  exploits sim symmetry: the baseline recomputed the neg-row x spos-col
  blocks (25.2M exps); here 16.8M only.

Sharding: 2D grid, 4 row-groups x 2 col-groups.  Core (r,g) owns 512
pos-pair rows (spos/wpos slab r) and 1024 neg cols (sneg/wneg group g,
rolled by -256r so each core owns a disjoint 256-slice of neg pairs).
Columns per core, in six 512-col chunks:
  [spos | sn0 | sn1 | wn0 | wn1 | wpos]  (3072 total)

Device pipeline:
  - 6 chunk DMAs, one descriptor per partition (host packs each chunk
    [h0|h1]-contiguous), same queue, NO dep chains: packets stream
    back-to-back at full BW and chunk completions stagger for the
    square chase.  iv/obb ride the gpsimd queue (Pool triggers ~25ns).
  - Column sumsq: DVE squares + h-presum -> ONE ones-window matmul per
    chunk (OZ1, value 1.0 - tau is NOT in the lhsT so nothing waits the
    iv DMA).  rn = Exp(-0.5*Ln(ssq) - 0.5*ln(tau)): Ln and Exp need one
    table load each way, the exp-set load sits between them
    (structural); dummy Ln at t=0 preloads the ln set during DMA.  The
    tau bias is a [6,1] AP built from a K=1 matmul + ACT Ln + mul.
  - rn broadcast WITHOUT DMA: one-hot [6x128] matmuls (OBB) replicate
    rn_t row c across partitions into [128,512] psum pieces; DVE
    normalizes zt against psum directly, writing fp8e4 ztn (the PSUM
    operand forces DVE 1x anyway, so fp8 output is free).
  - 16 main tiles [128,1024]: 2 fp8 DoubleRow matmuls each (K=256 in
    one pass via the [128,2,512] h-layout) + ACT Exp with fused row-sum
    (accum_out -> ACC[128,16]).  spos exps write real fp8 values; ONE
    DoubleRow window matmul per spos tile (OZD: kt0 hot at row 2u, kt1
    at 2u+1) col-sums both 512-chunks into cs rows 0..3.  Colsums are
    deferred into the wpos phase so the PE is never the per-tile
    bottleneck while its clock ramps.  Pair logits land in cs rows 4/5.
  - Outputs: ACC [128,16] and cs[0:6] f32; host finishes in f64.
  - PSUM: sps 1 bank (warm/ssq/cs) + rnb 2x1 + mains 2x2 = 7 of 8.
"""

import numpy as np

B = 4096
D = 256
P = 2048
NCORES = 8
RG = 4                    # row groups (pos-pair slabs of 512)
CG = 2                    # col groups (neg slabs of 1024)
SLAB = P // RG            # 512 pos pairs per row-group
CGN = P // CG             # 1024 negs per col-group
NCH = 6                   # 512-col chunks: [spos|sn0|sn1|wn0|wn1|wpos]
NCOL = 512 * NCH

_CACHE: dict = {}


def _build_nc():
    import concourse.bacc as bacc
    import concourse.tile as tile
    from concourse import mybir

    f32 = mybir.dt.float32
    f16 = mybir.dt.float16
    f8 = mybir.dt.float8e4
    DR = mybir.MatmulPerfMode.DoubleRow
    EXP = mybir.ActivationFunctionType.Exp
    LN = mybir.ActivationFunctionType.Ln

    SQUARE = mybir.ActivationFunctionType.Square

    nc = bacc.Bacc("TRN2", target_bir_lowering=False, debug=False,
                   num_devices=NCORES)
    zt_d = nc.dram_tensor("zt", [128, 2 * NCOL], f8, kind="ExternalInput")
    iv_d = nc.dram_tensor("tauv", [1, 1], f32, kind="ExternalInput")
    obb_d = nc.dram_tensor("obb", [6, 6 * 128], f16, kind="ExternalInput")
    acc_d = nc.dram_tensor("acc", [128, 16], f32, kind="ExternalOutput")
    cs_d = nc.dram_tensor("cs", [6, 512], f32, kind="ExternalOutput")

    # chunk-major host layout: row p = [c0h0|c0h1|c1h0|c1h1|...]
    zt6 = zt_d[:, :].rearrange("p (c h n) -> p c h n", c=NCH, h=2)

    with tile.TileContext(nc) as tc:
        with (
            tc.tile_pool(name="const", bufs=1) as constp,
            tc.tile_pool(name="big", bufs=1) as bigp,
            tc.tile_pool(name="work", bufs=3) as workp,
            tc.tile_pool(name="esc", bufs=8) as escp,
            tc.tile_pool(name="sps", bufs=1, space="PSUM") as sps,
            tc.tile_pool(name="rnbp", bufs=3, space="PSUM") as rnbp,
            tc.tile_pool(name="mps", bufs=2, space="PSUM") as mps,
        ):
            # ---------------- input DMAs (two queues, unchained) -------
            # fp8 input halves the stream time; even chunks trigger on
            # sync, odd on the scalar queue (ahead of the dummy Ln and
            # its table load) so all six triggers land early.
            # three trigger queues so the six chunk transfers + iv/obb
            # all start early; queue tails land mid-sequence
            zt4 = bigp.tile([128, NCH, 2, 512], f8)
            for c in (5, 3):
                nc.scalar.dma_start(zt4[:, c], zt6[:, c])
            for c in (0, 2):
                nc.sync.dma_start(zt4[:, c], zt6[:, c])

            # ---------------- gpsimd: dummy + aux DMAs + consts --------
            dum1 = constp.tile([1, 1], f16)
            nc.gpsimd.memset(dum1[:], 1.0)
            ivt = constp.tile([1, 1], f32)
            nc.gpsimd.dma_start(ivt[:], iv_d[:])
            nc.gpsimd.dma_start(zt4[:, 4], zt6[:, 4])
            nc.gpsimd.dma_start(zt4[:, 1], zt6[:, 1])
            OBB = constp.tile([6, 6 * 128], f16)
            nc.gpsimd.dma_start(OBB[:], obb_d[:])

            # dummy Ln: preloads the natural_log table set during DMA
            dumo = constp.tile([1, 1], f32)
            nc.scalar.activation(dumo[:], dum1[:], LN)

            ones16_1 = constp.tile([1, 128], f16)
            nc.gpsimd.memset(ones16_1[:], 1.0)
            # OZ1: ones window (ssq reductions + pair sums), hot col 6
            OZ1 = constp.tile([128, 12], f16)
            nc.gpsimd.memset(OZ1[:], 0.0)
            nc.gpsimd.memset(OZ1[:, 6:7], 1.0)
            # OZD: DoubleRow colsum windows, M=32 pad (DR rejects tiny
            # M): kt=0 hot at window pos m0, kt=1 at m0+1
            OZD = constp.tile([128, 128], f8)
            nc.gpsimd.memset(OZD[:], 0.0)
            nc.gpsimd.memset(OZD[:, 32:33], 1.0)
            nc.gpsimd.memset(OZD[:, 97:98], 1.0)
            OZD3 = OZD[:, :].rearrange("p (k c) -> p k c", k=2)
            # OZDS: both k-tiles hot at the SAME row -> one DR matmul
            # h-sums a [128,2,512] fp8 square chunk into ssq row c
            OZDS = constp.tile([128, 128], f8)
            nc.gpsimd.memset(OZDS[:], 0.0)
            nc.gpsimd.memset(OZDS[:, 32:33], 1.0)
            nc.gpsimd.memset(OZDS[:, 96:97], 1.0)
            OZDS3 = OZDS[:, :].rearrange("p (k c) -> p k c", k=2)

            # ---------------- tau bias: -0.5*ln(tau) as [6,1] ----------
            iv16 = constp.tile([1, 1], f16)
            nc.vector.tensor_copy(iv16[:], ivt[:])
            tau_bc = rnbp.tile([128, 512], f32, tag="rnb", name="tau_bc")
            nc.tensor.matmul(tau_bc[0:6, 0:1], ones16_1[0:1, 0:6],
                             iv16[0:1, 0:1], start=True, stop=True)
            lntau = constp.tile([6, 1], f32)
            nc.scalar.activation(lntau[:], tau_bc[0:6, 0:1], LN)
            bias_t = constp.tile([6, 1], f32)
            nc.scalar.mul(bias_t[:], lntau[:], -0.5)

            # ---------------- column sumsq -> rn ----------------
            # squares split across idle ACT (c0,c2,c4; Square is in the
            # natural_log set) and DVE (c5,c3,c1), written fp8 so ONE
            # DoubleRow window matmul per chunk h-sums straight into ssq
            # row c (no separate h-presum pass).
            ARR = (0, 5, 2, 3, 4, 1)
            sq4 = bigp.tile([128, NCH, 2, 512], f8)
            ssq = sps.tile([32, 512], f32, tag="sps")
            for c in (5, 3, 1):
                nc.vector.tensor_mul(sq4[:, c], zt4[:, c], zt4[:, c])
            for c in (0, 2, 4):
                nc.scalar.activation(sq4[:, c], zt4[:, c], SQUARE)
            for i, c in enumerate(ARR):
                nc.tensor.matmul(ssq[0:32, 0:512],
                                 OZDS3[:, :, 32 - c:64 - c], sq4[:, c],
                                 start=(i == 0), stop=(i == NCH - 1),
                                 perf_mode=DR)

            lnt = constp.tile([6, 512], f32)
            nc.scalar.activation(lnt[:], ssq[0:6, :], LN)
            rn_t = constp.tile([6, 512], f16)
            nc.scalar.activation(rn_t[:], lnt[:], EXP, scale=-0.5,
                                 bias=bias_t[:])

            # ---------------- rn broadcast + normalize (fp8) ----------
            ztn = bigp.tile([128, NCH, 2, 512], f8)
            for c in range(NCH):
                rp = rnbp.tile([128, 512], f32, tag="rnb", name=f"rnb{c}")
                nc.tensor.matmul(rp[:], OBB[:, 128 * c:128 * c + 128],
                                 rn_t[0:6, :], start=True, stop=True)
                for h in range(2):
                    nc.vector.tensor_mul(ztn[:, c, h], zt4[:, c, h], rp[:])

            # ---------------- main tiles ----------------
            ACC = constp.tile([128, 16], f32)
            escJ = constp.tile([128, 1024], f8)    # wpos exp sink
            cs = sps.tile([32, 512], f32, tag="sps")
            cs_started = [False]
            escs = []

            def cs_mm(row, rhs_ap, stop=False):
                nc.tensor.matmul(cs[0:6, 0:rhs_ap.shape[-1]],
                                 OZ1[:, 6 - row:12 - row], rhs_ap,
                                 start=not cs_started[0], stop=stop,
                                 skip_group_check=True)
                cs_started[0] = True

            def cs_mm_dr(u, esc, stop=False):
                # one DoubleRow matmul col-sums BOTH 512-chunks of an
                # esc tile into cs rows 2u (kt=0) and 2u+1 (kt=1)
                e3 = esc[:, :].rearrange("p (k n) -> p k n", k=2)
                m0 = 2 * u
                nc.tensor.matmul(cs[0:32, 0:512],
                                 OZD3[:, :, 32 - m0:64 - m0], e3[:, :, :],
                                 start=not cs_started[0], stop=stop,
                                 perf_mode=DR, skip_group_check=True)
                cs_started[0] = True

            def main_tile(T, lhs_ch, lhs_off, u, is_spos):
                ps = mps.tile([128, 1024], f32, tag="mps", name=f"mm{T}")
                for c2 in range(2):
                    nc.tensor.matmul(
                        ps[:, 512 * c2:512 * c2 + 512],
                        ztn[:, lhs_ch, :, lhs_off:lhs_off + 128],
                        ztn[:, 1 + 2 * u + c2], start=True, stop=True,
                        perf_mode=DR)
                if is_spos:
                    esc = escp.tile([128, 1024], f8, tag="esc",
                                    name=f"esc{T}")
                    nc.scalar.activation(esc[:], ps[:], EXP,
                                         accum_out=ACC[:, T:T + 1])
                    escs.append((esc, u))
                else:
                    nc.scalar.activation(escJ[:], ps[:], EXP,
                                         accum_out=ACC[:, T:T + 1])

            # all u=0 tiles first: they only need chunks 0..2 normalized
            for u in range(2):
                for t in range(4):
                    main_tile(2 * t + u, 0, 128 * t, u, True)

            # pair logits (DVE products + window matmuls into cs 4/5)
            pr_pos = workp.tile([128, 2, 512], f16, tag="pr")
            nc.vector.tensor_mul(pr_pos[:], ztn[:, 0], ztn[:, 5])
            pr_neg = workp.tile([128, 2, 256], f16, tag="pr")
            nc.vector.tensor_mul(pr_neg[:], ztn[:, 1, :, 0:256],
                                 ztn[:, 3, :, 0:256])

            k = [0]
            for u in range(2):
                for t in range(4):
                    main_tile(8 + 2 * t + u, 5, 128 * t, u, False)
                    e, eu = escs[k[0]]
                    cs_mm_dr(eu, e, stop=(k[0] == 7))
                    k[0] += 1
                    if k[0] == 4:
                        # pair-sum matmuls mid-phase so the cs copy+DMA
                        # stay off the measured tail
                        cs_mm(4, pr_pos[:, 0, :])
                        cs_mm(4, pr_pos[:, 1, :])
                        cs_mm(5, pr_neg[:, 0, :])
                        cs_mm(5, pr_neg[:, 1, :])

            # ---------------- outputs ----------------
            csb = constp.tile([6, 512], f32)
            nc.vector.tensor_copy(csb[:], cs[0:6, :])
            nc.sync.dma_start(acc_d[:], ACC[:])
            nc.sync.dma_start(cs_d[:], csb[:])

    nc.compile()
    return nc


def get_nc():
    if "nc" not in _CACHE:
        _CACHE["nc"] = _build_nc()
    return _CACHE["nc"]


def make_in_maps(strong: np.ndarray, weak: np.ndarray, temp: np.ndarray):
    """Host-side sharding: slice + roll + transpose (pure data movement)."""
    tauv = np.asarray(temp, np.float32).reshape(1, 1)
    obb = np.zeros((6, 6 * 128), np.float16)
    for j in range(NCH):
        obb[j, 128 * j:128 * j + 128] = 1.0
    in_maps = []
    for r in range(RG):
        for g in range(CG):
            spos = strong[SLAB * r:SLAB * r + SLAB]
            wpos = weak[SLAB * r:SLAB * r + SLAB]
            sneg = np.roll(strong[P + CGN * g:P + CGN * g + CGN],
                           -256 * r, axis=0)
            wneg = np.roll(weak[P + CGN * g:P + CGN * g + CGN],
                           -256 * r, axis=0)
            import ml_dtypes
            cols = np.concatenate([spos, sneg, wneg, wpos], axis=0)
            zt8 = cols.T.astype(ml_dtypes.float8_e4m3fn)  # [256, 3072]
            # [h,p,c,n] -> [p, c, h, n] chunk-major contiguous rows
            ztd = np.ascontiguousarray(
                zt8.reshape(2, 128, NCH, 512).transpose(1, 2, 0, 3)
                .reshape(128, 2 * NCOL))
            in_maps.append({"zt": ztd, "tauv": tauv, "obb": obb})
    return in_maps


def kernel(inputs, strong_inputs, targets, num_pos, temperature):
    assert int(num_pos) == P
    strong = np.ascontiguousarray(np.asarray(strong_inputs, dtype=np.float32))
    weak = np.ascontiguousarray(np.asarray(inputs, dtype=np.float32))
    temp = np.asarray(temperature, dtype=np.float32).reshape(1, 1)

    from concourse.bass_utils import run_bass_kernel_spmd

    nc = get_nc()
    in_maps = make_in_maps(strong, weak, temp)
    res = run_bass_kernel_spmd(nc, in_maps, core_ids=list(range(NCORES)))
    return finish_host(res.results)


def finish_host(results):
    """Final ln(S + e^p) - p reduction in float64 on the host."""
    S1s = np.zeros((RG, SLAB))
    S1w = np.zeros((RG, SLAB))
    CA = np.zeros((CG, CGN))
    CB = np.zeros((CG, CGN))
    pos_l = np.zeros((RG, SLAB))
    neg_l = np.zeros((CG, CGN))
    for r in range(RG):
        for g in range(CG):
            res = results[CG * r + g]
            acc = np.asarray(res["acc"], np.float64)     # [128, 16]
            cs = np.asarray(res["cs"], np.float64)       # [6, 512]
            for t in range(4):
                sl = slice(128 * t, 128 * t + 128)
                S1s[r, sl] += acc[:, 2 * t] + acc[:, 2 * t + 1]
                S1w[r, sl] += acc[:, 8 + 2 * t] + acc[:, 8 + 2 * t + 1]
            CA[g] += np.roll(cs[0:2].reshape(CGN), 256 * r)
            CB[g] += np.roll(cs[2:4].reshape(CGN), 256 * r)
            if g == 0:
                pos_l[r] = cs[4]
            neg_l[g, 256 * r:256 * r + 256] = cs[5, 0:256]
    p = pos_l.reshape(-1)
    q = neg_l.reshape(-1)
    ep, eq = np.exp(p), np.exp(q)
    total = (np.sum(np.log(S1s.reshape(-1) + ep) - p)
             + np.sum(np.log(S1w.reshape(-1) + ep) - p)
             + np.sum(np.log(CA.reshape(-1) + eq) - q)
             + np.sum(np.log(CB.reshape(-1) + eq) - q))
    return np.float32(total / (2 * B))
